# revision 5
# baseline (speedup 1.0000x reference)
"""Trainium2 Bass kernel for the DAN classifier (gather + segment-mean + MLP).

Full computation:
    gathered = embeddings[docs]                    # [B, L, D]
    avg = gathered.sum(1) / doc_lens[:, None]      # [B, D]
    out = relu(relu(avg @ W1 + b1) @ W2 + b2) @ W3 + b3   # [B, C]

Sharding: data-parallel over 8 cores, 32 docs each; embedding table and MLP
weights replicated. Each core gathers its 32,000 embedding rows with the
GPSIMD dma_gather (Ant) instruction, which needs int16 indices and rows that
are a multiple of 256 bytes. So the host:
  - splits each fp32 embedding row into bf16 hi/lo halves (x = hi + lo to
    ~16-bit mantissa) and packs them into one [100000, 640]-bf16 row
    (hi[0:300] | lo[300:600] | zero pad) = 1280 B, same traffic as padded
    fp32 but matmul-able on the PE at 1 cycle/row;
  - splits the vocab into 4 shards of 25000 rows (int16-addressable) and
    buckets each core's tokens by shard (vocab-sorted for HBM locality),
    padding each bucket to a fixed 8704 capacity with index 0 / doc_id -1.

Per gathered [128, 640] token tile the device builds a [128, 32] one-hot
doc-selection matrix from the streamed doc ids (is_equal against an iota row;
pad slots with doc_id -1 give an all-zero row, masking the garbage), then two
bf16 matmuls (hi, lo halves) accumulate doc sums into one fp32 PSUM tile.
The segment-mean and the 3-layer MLP run on PE/ACT in fp32 (exact).
"""

import numpy as np

# Problem shapes (hardcoded per contract).
V, D = 100000, 300
B, L = 256, 1000
H, C = 512, 5
NCORES = 8
BC = B // NCORES            # docs per core = 32
NSHARD = 4                  # vocab shards (int16 index limit)
VS = V // NSHARD            # 25000 rows per shard
DPACK = 640                 # packed bf16 row: hi 300 | lo 300 | pad 40
CAP = 9216                  # per-(core, shard) token capacity (72 tiles)
NTS = CAP // 128            # tiles per shard = 72
TCH = 8                     # tiles per dma_gather call (<=1024 idx: SWDGE ring)
NCH = NTS // TCH            # gather calls per shard = 9
NIDX = TCH * 128            # indices per gather call = 1024
DCH = 100                   # D split for transposes / W1 chunks (3 x 100)

_CACHE = {}


def _build_nc():
    import concourse.bass as bass
    import concourse.bacc as bacc
    import concourse.mybir as mybir
    import concourse.tile as tile

    dt = mybir.dt
    f32 = dt.float32
    bf16 = dt.bfloat16

    nc = bacc.Bacc("TRN2", target_bir_lowering=False, debug=False)

    ehl_d = [
        nc.dram_tensor(f"ehl{s_}", [VS, DPACK], bf16, kind="ExternalInput")
        for s_ in range(NSHARD)
    ]
    idx_d = nc.dram_tensor("idxs", [128, NSHARD * CAP // 16], dt.int16, kind="ExternalInput")
    did_d = nc.dram_tensor("dids", [128, NSHARD * NTS], dt.int16, kind="ExternalInput")
    iota_d = nc.dram_tensor("iota", [128, BC], dt.int16, kind="ExternalInput")
    invl_d = nc.dram_tensor("invl", [BC, 1], f32, kind="ExternalInput")
    w1_d = nc.dram_tensor("w1", [DCH, 3 * H], f32, kind="ExternalInput")
    w2_d = nc.dram_tensor("w2", [128, 4 * H], f32, kind="ExternalInput")
    w3_d = nc.dram_tensor("w3", [128, 4 * C], f32, kind="ExternalInput")
    b1_d = nc.dram_tensor("b1", [128, 4], f32, kind="ExternalInput")
    b2_d = nc.dram_tensor("b2", [128, 4], f32, kind="ExternalInput")
    b3_d = nc.dram_tensor("b3", [1, C], f32, kind="ExternalInput")
    ones_d = nc.dram_tensor("ones", [1, BC], f32, kind="ExternalInput")
    ident_d = nc.dram_tensor("ident", [BC, BC], f32, kind="ExternalInput")
    out_d = nc.dram_tensor("out", [BC, C], f32, kind="ExternalOutput")

    relu = mybir.ActivationFunctionType.Relu
    SCOL = CAP // 16  # idx columns per shard (544)

    with tile.TileContext(nc) as tc:
        with (
            tc.tile_pool(name="const", bufs=1) as cp,
            tc.tile_pool(name="gather", bufs=3) as gp,
            tc.tile_pool(name="selp", bufs=4) as sp,
            tc.tile_pool(name="work", bufs=1) as wp,
            tc.tile_pool(name="psacc", bufs=1, space="PSUM") as pp,
            tc.tile_pool(name="psmlp", bufs=3, space="PSUM") as pp2,
        ):
            idx_sb = cp.tile([128, NSHARD * SCOL], dt.int16)
            nc.sync.dma_start(out=idx_sb[:], in_=idx_d[:])
            did_sb = cp.tile([128, NSHARD * NTS], dt.int16)
            nc.sync.dma_start(out=did_sb[:], in_=did_d[:])
            iota_sb = cp.tile([128, BC], dt.int16)
            nc.sync.dma_start(out=iota_sb[:], in_=iota_d[:])
            invl_sb = cp.tile([BC, 1], f32)
            nc.sync.dma_start(out=invl_sb[:], in_=invl_d[:])
            w1_sb = cp.tile([DCH, 3 * H], f32)
            nc.sync.dma_start(out=w1_sb[:], in_=w1_d[:])
            w2_sb = cp.tile([128, 4 * H], f32)
            nc.sync.dma_start(out=w2_sb[:], in_=w2_d[:])
            w3_sb = cp.tile([128, 4 * C], f32)
            nc.sync.dma_start(out=w3_sb[:], in_=w3_d[:])
            b1_sb = cp.tile([128, 4], f32)
            nc.sync.dma_start(out=b1_sb[:], in_=b1_d[:])
            b2_sb = cp.tile([128, 4], f32)
            nc.sync.dma_start(out=b2_sb[:], in_=b2_d[:])
            b3_sb = cp.tile([1, C], f32)
            nc.sync.dma_start(out=b3_sb[:], in_=b3_d[:])
            ones_sb = cp.tile([1, BC], f32)
            nc.sync.dma_start(out=ones_sb[:], in_=ones_d[:])
            ident_sb = cp.tile([BC, BC], f32)
            nc.sync.dma_start(out=ident_sb[:], in_=ident_d[:])

            psum_doc = pp.tile([BC, D], f32)

            first = True
            for s in range(NSHARD):
                for ch in range(NCH):
                    gt = gp.tile([128, TCH, DPACK], bf16)
                    nc.gpsimd.dma_gather(
                        out_ap=gt[:],
                        in_ap=ehl_d[s][:],
                        idxs_ap=idx_sb[
                            :, s * SCOL + ch * (NIDX // 16) : s * SCOL + (ch + 1) * (NIDX // 16)
                        ],
                        num_idxs=NIDX,
                        num_idxs_reg=NIDX,
                        elem_size=DPACK,
                    )
                    for t in range(TCH):
                        tg = s * NTS + ch * TCH + t
                        last = (s == NSHARD - 1) and (ch == NCH - 1) and (t == TCH - 1)
                        sel = sp.tile([128, BC], bf16)
                        nc.vector.tensor_tensor(
                            out=sel[:],
                            in0=did_sb[:, tg : tg + 1].to_broadcast([128, BC]),
                            in1=iota_sb[:],
                            op=mybir.AluOpType.is_equal,
                        )
                        nc.tensor.matmul(
                            out=psum_doc[:],
                            lhsT=sel[:],
                            rhs=gt[:, t, 0:D],
                            start=first,
                            stop=False,
                        )
                        first = False
                        nc.tensor.matmul(
                            out=psum_doc[:],
                            lhsT=sel[:],
                            rhs=gt[:, t, D : 2 * D],
                            start=False,
                            stop=last,
                        )

            # Mean: divide by doc length (per-partition scalar).
            avg = wp.tile([BC, D], f32)
            nc.vector.tensor_scalar_mul(avg[:], psum_doc[:], invl_sb[:])

            # Transpose to [D, BC] in three 100-column chunks.
            avgT = wp.tile([DCH, 3 * BC], f32)
            for c3 in range(3):
                pt = pp2.tile([DCH, BC], f32, tag="mlp")
                nc.tensor.transpose(
                    out=pt[:],
                    in_=avg[:, c3 * DCH : (c3 + 1) * DCH],
                    identity=ident_sb[:],
                )
                nc.vector.tensor_copy(out=avgT[:, c3 * BC : (c3 + 1) * BC], in_=pt[:])

            # Layer 1: h1T[j] = relu(W1[:, j-chunk]^T @ avgT + b1), j over 4x128.
            h1 = wp.tile([128, 4 * BC], f32)
            for j in range(4):
                p1 = pp2.tile([128, BC], f32, tag="mlp")
                for c3 in range(3):
                    nc.tensor.matmul(
                        out=p1[:],
                        lhsT=w1_sb[:, c3 * H + 128 * j : c3 * H + 128 * j + 128],
                        rhs=avgT[:, c3 * BC : (c3 + 1) * BC],
                        start=(c3 == 0),
                        stop=(c3 == 2),
                    )
                nc.scalar.activation(
                    out=h1[:, j * BC : (j + 1) * BC],
                    in_=p1[:],
                    func=relu,
                    bias=b1_sb[:, j : j + 1],
                )

            # Layer 2: h2T[j] = relu(sum_k W2[k-chunk, j-chunk]^T @ h1T[k] + b2).
            h2 = wp.tile([128, 4 * BC], f32)
            for j in range(4):
                p2 = pp2.tile([128, BC], f32, tag="mlp")
                for k in range(4):
                    nc.tensor.matmul(
                        out=p2[:],
                        lhsT=w2_sb[:, k * H + 128 * j : k * H + 128 * j + 128],
                        rhs=h1[:, k * BC : (k + 1) * BC],
                        start=(k == 0),
                        stop=(k == 3),
                    )
                nc.scalar.activation(
                    out=h2[:, j * BC : (j + 1) * BC],
                    in_=p2[:],
                    func=relu,
                    bias=b2_sb[:, j : j + 1],
                )

            # Layer 3: out = sum_j h2T[j]^T @ W3[j-chunk] + b3 (bias via K=1 matmul).
            pout = pp2.tile([BC, C], f32, tag="mlp")
            for j in range(4):
                nc.tensor.matmul(
                    out=pout[:],
                    lhsT=h2[:, j * BC : (j + 1) * BC],
                    rhs=w3_sb[:, j * C : (j + 1) * C],
                    start=(j == 0),
                    stop=False,
                )
            nc.tensor.matmul(
                out=pout[:], lhsT=ones_sb[:], rhs=b3_sb[:], start=False, stop=True
            )

            out_sb = wp.tile([BC, C], f32)
            nc.vector.tensor_copy(out=out_sb[:], in_=pout[:])
            nc.sync.dma_start(out=out_d[:], in_=out_sb[:])

    nc.finalize()
    return nc


def _get_nc():
    if "nc" not in _CACHE:
        _CACHE["nc"] = _build_nc()
    return _CACHE["nc"]


def _pack_table(embeddings):
    import ml_dtypes

    emb = np.asarray(embeddings, np.float32)
    hi = emb.astype(ml_dtypes.bfloat16)
    lo = (emb - hi.astype(np.float32)).astype(ml_dtypes.bfloat16)
    ehl = np.zeros((V, DPACK), ml_dtypes.bfloat16)
    ehl[:, :D] = hi
    ehl[:, D : 2 * D] = lo
    return ehl


def make_in_maps(embeddings, W1, b1, W2, b2, W3, b3, docs, doc_lens):
    """Host-side sharding/permutation. Returns one input dict per core."""
    ehl = _pack_table(embeddings)
    w1 = np.ascontiguousarray(
        np.asarray(W1, np.float32).reshape(3, DCH, H).transpose(1, 0, 2).reshape(DCH, 3 * H)
    )
    w2 = np.ascontiguousarray(
        np.asarray(W2, np.float32).reshape(4, 128, H).transpose(1, 0, 2).reshape(128, 4 * H)
    )
    w3 = np.ascontiguousarray(
        np.asarray(W3, np.float32).reshape(4, 128, C).transpose(1, 0, 2).reshape(128, 4 * C)
    )
    b1p = np.ascontiguousarray(np.asarray(b1, np.float32).reshape(4, 128).T)
    b2p = np.ascontiguousarray(np.asarray(b2, np.float32).reshape(4, 128).T)
    b3r = np.ascontiguousarray(np.asarray(b3, np.float32).reshape(1, C))
    ones = np.ones((1, BC), np.float32)
    ident = np.eye(BC, dtype=np.float32)
    iota = np.ascontiguousarray(np.tile(np.arange(BC, dtype=np.int16), (128, 1)))

    docs = np.asarray(docs, np.int32)
    doc_lens = np.asarray(doc_lens, np.int32)

    in_maps = []
    for core in range(NCORES):
        v = docs[core * BC : (core + 1) * BC].ravel()        # [32000]
        dof = np.arange(v.size, dtype=np.int32) // L          # doc of token
        shard = v // VS
        idx_w = np.zeros((128, NSHARD * CAP // 16), np.int16)
        did_w = np.full((128, NSHARD * NTS), -1, np.int16)
        for s in range(NSHARD):
            m = shard == s
            vals = (v[m] - s * VS).astype(np.int16)
            ds = dof[m].astype(np.int16)
            if vals.size > CAP:
                raise ValueError(
                    f"shard bucket overflow: core {core} shard {s} has "
                    f"{vals.size} tokens > capacity {CAP}"
                )
            order = np.argsort(vals, kind="stable")           # vocab locality
            vals, ds = vals[order], ds[order]
            pv = np.zeros(CAP, np.int16)
            pv[: vals.size] = vals
            pd = np.full(CAP, -1, np.int16)
            pd[: ds.size] = ds
            # index i -> partition i%16, column i//16 (replicated x8 groups)
            idx_w[:16, s * (CAP // 16) : (s + 1) * (CAP // 16)] = pv.reshape(
                CAP // 16, 16
            ).T
            # doc id of slot i -> partition i%128, column i//128
            did_w[:, s * NTS : (s + 1) * NTS] = pd.reshape(NTS, 128).T
        idx_w[16:] = np.tile(idx_w[:16], (7, 1))
        invl = (1.0 / doc_lens[core * BC : (core + 1) * BC].astype(np.float32)).reshape(
            BC, 1
        )
        in_maps.append(
            {
                **{f"ehl{s_}": ehl[s_ * VS : (s_ + 1) * VS] for s_ in range(NSHARD)},
                "idxs": np.ascontiguousarray(idx_w),
                "dids": np.ascontiguousarray(did_w),
                "iota": iota,
                "invl": np.ascontiguousarray(invl),
                "w1": w1,
                "w2": w2,
                "w3": w3,
                "b1": b1p,
                "b2": b2p,
                "b3": b3r,
                "ones": ones,
                "ident": ident,
            }
        )
    return in_maps


def kernel(embeddings, W1, b1, W2, b2, W3, b3, docs, doc_lens):
    from concourse.bass_utils import run_bass_kernel_spmd

    nc = _get_nc()
    in_maps = make_in_maps(embeddings, W1, b1, W2, b2, W3, b3, docs, doc_lens)
    res = run_bass_kernel_spmd(nc, in_maps, list(range(NCORES)))
    out = np.concatenate([res.results[i]["out"] for i in range(NCORES)], axis=0)
    return out.astype(np.float32)
